# revision 22
# baseline (speedup 1.0000x reference)
"""Trainium2 Bass kernel for nn_Attention_27376121544790.

One batch element per NeuronCore (B=8 -> 8 cores, no collectives).

Math per core (transposed [feature, token] device layout):
  qk   = x @ W.T + b                         [N, D] (kept as qkT [D, N])
  q = k = l2norm(qk per 64-dim head)
  E    = exp((q @ q.T) * sqrt(64)/attn_gamma)  per head, dense softmax numerator
  Z    = column sums of E (ones-column in the augmented v)
  out  = (E.T @ v) / Z
  final= w0*(out @ W.T) + w1*qk + w0*b       since x @ W.T == qk - b (gamma==0)

Speed levers vs the bf16 baseline:
  * fp8e4 DoubleRow matmuls (0.5 cyc/row, 2x contraction per instr).  The
    projection runs 3 hi/lo-split terms (xh@Wh + xl@Wh + xh@Wl) because its
    result feeds the output directly through the w1*qk reuse term; the gram
    uses a zero second plane (contraction is only 64); attn@v splits v hi+lo.
    Host pre-scales x by 8 and W by 64 to dodge fp8 denormals; the
    projection drain descales.
  * exp work is split within each head across ACT (table exp -> fp8) and DVE
    (Schraudolph: i8 = round(g*A + B) bitcast to fp8e4) -- the two engines
    that can read PSUM.  GPSIMD cannot touch PSUM; it gets SBUF-only work
    (qn normalize, out scaling, w1*qk).  Squares for the norms run on ACT
    (Square needs no table load).
  * half-partition broadcasts (invn, w0/Z) via 0-stride-source DMAs instead
    of PE broadcast matmuls, so their consumers can run on GPSIMD.
  * the final blend collapses into the projection drain via the qk reuse
    identity; Z for chunks 0-6 is processed while heads 14/15 still run, so
    the final projection overlaps the attention tail.
"""

import math
import os

import numpy as np

B, N, C, D = 8, 1024, 1024, 1024
HEADS, HD = 16, 64
P = 128
EPS = 1e-6
NCHUNK = C // P   # 8 feature chunks of 128
FH = 512          # free-dim half (PSUM bank width in f32)
XS, WS, QS = 8.0, 64.0, 8.0   # fp8 pre-scales for x, W, qn
LOG2E = 1.0 / math.log(2.0)


def _build(gamma: float, w0: float, w1: float, logit_scale: float):
    import concourse.bass as bass
    import concourse.tile as tile
    from concourse import bacc, mybir

    f32 = mybir.dt.float32
    BF16 = mybir.dt.bfloat16
    FP8 = mybir.dt.float8e4
    I8 = mybir.dt.int8
    DR = mybir.MatmulPerfMode.DoubleRow

    Exp = mybir.ActivationFunctionType.Exp
    Ln = mybir.ActivationFunctionType.Ln
    Square = mybir.ActivationFunctionType.Square
    Copy = mybir.ActivationFunctionType.Copy
    MULT = mybir.AluOpType.mult
    ADD = mybir.AluOpType.add

    # per-head exp-engine split: DVE handles these mb indices (rest ACT)
    DVE_MB_EVEN = frozenset({2, 4, 6})
    DVE_MB_ODD = frozenset({1, 3, 5, 7})
    # stage eviction: heads with h%4 < SDVE4 evict via DVE, rest via ACT
    SDVE4 = int(os.environ.get("BK_SDVE4", "1"))

    has_pos = gamma != 0.0

    ls8 = logit_scale / (QS * QS)
    sch_a = ls8 * 8.0 * LOG2E
    sch_b = 56.0 - 8.0 * 0.0434

    nc = bacc.Bacc("TRN2", target_bir_lowering=False, debug=False)

    xdrh_d = nc.declare_dram_parameter("xdrh", [4 * P, 2 * N], FP8, isOutput=False)
    xdrl_d = nc.declare_dram_parameter("xdrl", [4 * P, 2 * N], FP8, isOutput=False)
    wdrh_d = nc.declare_dram_parameter("wdrh", [4 * P, 2 * D], FP8, isOutput=False)
    wdrl_d = nc.declare_dram_parameter("wdrl", [4 * P, 2 * D], FP8, isOutput=False)
    xah_d = nc.declare_dram_parameter("xah", [4 * P, 2 * HEADS * (HD + 1)], FP8, isOutput=False)
    xal_d = nc.declare_dram_parameter("xal", [4 * P, 2 * HEADS * (HD + 1)], FP8, isOutput=False)
    wt_d = nc.declare_dram_parameter("wt", [C, D], BF16, isOutput=False)
    bd8_d = nc.declare_dram_parameter("bd8", [C, 8], FP8, isOutput=False)
    bmat_d = nc.declare_dram_parameter("bmat", [P, NCHUNK], f32, isOutput=False)
    bpr_d = nc.declare_dram_parameter("bpr", [P, NCHUNK], f32, isOutput=False)
    if has_pos:
        xbf_d = nc.declare_dram_parameter("xbf", [C, N], BF16, isOutput=False)
    out_d = nc.declare_dram_parameter("out", [D, N], f32, isOutput=True)

    AW = HEADS * (HD + 1)  # 1040, xaug free width per plane

    def r2(ap):
        return ap.rearrange("p (two n) -> p two n", two=2)

    with tile.TileContext(nc) as tc:
        with (
            tc.tile_pool(name="persist", bufs=1) as pers,
            tc.tile_pool(name="small", bufs=1) as small,
        ):
            qkT_t = [pers.tile([P, N], f32, tag=f"qk{c}", name=f"qk{c}") for c in range(NCHUNK)]
            qn_t = [pers.tile([P, 2 * N], FP8, tag=f"qn{c}", name=f"qn{c}") for c in range(NCHUNK)]
            out_t = [pers.tile([P, N], BF16, tag=f"ot{c}", name=f"ot{c}") for c in range(NCHUNK)]
            wt_t = [pers.tile([P, D], BF16, tag=f"wt{c}", name=f"wt{c}") for c in range(NCHUNK)]
            xah_t = [pers.tile([P, 2 * AW], FP8, tag=f"xh{j}", name=f"xh{j}") for j in range(4)]
            xal_t = [pers.tile([P, 2 * AW], FP8, tag=f"xl{j}", name=f"xl{j}") for j in range(4)]
            bd8_t = [small.tile([P, 8], FP8, tag=f"bd8{c}", name=f"bd8{c}") for c in range(NCHUNK)]
            if has_pos:
                xbf_t = [pers.tile([P, N], BF16, tag=f"xb{c}", name=f"xb{c}") for c in range(NCHUNK)]
            bmat_t = small.tile([P, NCHUNK], f32, tag="bmat", name="bmat")
            bpr_t = small.tile([P, NCHUNK], f32, tag="bpr", name="bpr")
            invnA_t = small.tile([8, N], f32, tag="invnA", name="invnA")
            invnB_t = small.tile([8, N], f32, tag="invnB", name="invnB")
            lns_t = small.tile([8, N], f32, tag="lns", name="lns")
            zall_t = small.tile([HEADS, N], BF16, tag="zall", name="zall")
            zallf_t = small.tile([HEADS, N], f32, tag="zallf", name="zallf")
            rz_t = small.tile([HEADS, N], f32, tag="rz", name="rz")
            eps_t = small.tile([HEADS, 1], f32, tag="eps", name="eps")
            dummy_t = small.tile([1, 16], f32, tag="dummy", name="dummy")

            # preload the ln/exp ACT table set during input DMA; ln and exp
            # share set 6 so no further table swaps happen
            nc.gpsimd.memset(dummy_t[:], 1.0)
            nc.scalar.activation(dummy_t[:], dummy_t[:], Ln)
            nc.gpsimd.memset(eps_t[:], EPS / (QS * QS))
            for c in range(NCHUNK):
                nc.gpsimd.memset(qn_t[c][:, N:2 * N], 0.0)

            with (
                tc.tile_pool(name="dr_in", bufs=1) as pdr,
                tc.tile_pool(name="psA", bufs=3, space="PSUM") as psA,
                tc.tile_pool(name="sq", bufs=5) as psq,
                tc.tile_pool(name="bcast", bufs=2) as pbc,
                tc.tile_pool(name="E", bufs=8) as pE,
                tc.tile_pool(name="stage", bufs=3) as pstage,
                tc.tile_pool(name="zb", bufs=2) as pzb,
            ):
                wdh_t = [pdr.tile([P, 2 * D], FP8, tag=f"wdh{k}", name=f"wdh{k}") for k in range(4)]
                xdh_t = [pdr.tile([P, 2 * N], FP8, tag=f"xdh{k}", name=f"xdh{k}") for k in range(4)]
                wdl_t = [pdr.tile([P, 2 * D], FP8, tag=f"wdl{k}", name=f"wdl{k}") for k in range(4)]
                xdl_t = [pdr.tile([P, 2 * N], FP8, tag=f"xdl{k}", name=f"xdl{k}") for k in range(4)]
                for k in range(4):
                    nc.sync.dma_start(wdh_t[k][:], wdrh_d[k * P:(k + 1) * P, :])
                    nc.sync.dma_start(xdh_t[k][:], xdrh_d[k * P:(k + 1) * P, :])
                for k in range(4):
                    nc.sync.dma_start(xdl_t[k][:], xdrl_d[k * P:(k + 1) * P, :])
                    nc.sync.dma_start(wdl_t[k][:], wdrl_d[k * P:(k + 1) * P, :])
                for c in range(NCHUNK):
                    nc.sync.dma_start(bd8_t[c][:], bd8_d[c * P:(c + 1) * P, :])
                nc.sync.dma_start(bmat_t[:], bmat_d[:])
                nc.sync.dma_start(bpr_t[:], bpr_d[:])
                for j in range(4):
                    nc.sync.dma_start(xah_t[j][:], xah_d[j * P:(j + 1) * P, :])
                for j in range(4):
                    nc.sync.dma_start(xal_t[j][:], xal_d[j * P:(j + 1) * P, :])
                for c in range(NCHUNK):
                    nc.sync.dma_start(wt_t[c][:], wt_d[c * P:(c + 1) * P, :])
                if has_pos:
                    for c in range(NCHUNK):
                        nc.sync.dma_start(xbf_t[c][:], xbf_d[c * P:(c + 1) * P, :])

                terms = [(wdh_t, xdh_t), (wdh_t, xdl_t), (wdl_t, xdh_t)]
                sq_t = [None] * NCHUNK
                pav_pool = None

                def proj_chunk(m, ssq_ps):
                    """proj1 chunk m (3-term hi/lo DR) + drain + square + its
                    ssq contribution (rows 2*(m%4) of the batch psum)."""
                    ps = psA.tile([P, N], f32, tag="pg", name="pg")
                    for fn in range(2):
                        first = True
                        for wt_dr, xt_dr in terms:
                            for kk in range(4):
                                nc.tensor.matmul(
                                    ps[:, fn * FH:(fn + 1) * FH],
                                    r2(wt_dr[kk][:])[:, :, m * P:(m + 1) * P],
                                    r2(xt_dr[kk][:])[:, :, fn * FH:(fn + 1) * FH],
                                    start=first,
                                    stop=(wt_dr is wdl_t and kk == 3),
                                    perf_mode=DR)
                                first = False
                    nc.vector.tensor_scalar(
                        out=qkT_t[m][:], in0=ps[:], scalar1=1.0 / (XS * WS),
                        scalar2=bmat_t[:, m:m + 1], op0=MULT, op1=ADD)
                    sq = psq.tile([P, N], FP8, tag="sq", name="sq")
                    sq_t[m] = sq
                    nc.scalar.activation(sq[:], qkT_t[m][:], Square)
                    if ssq_ps is not None:
                        for fn in range(2):
                            nc.tensor.matmul(
                                ssq_ps[:, fn * FH:(fn + 1) * FH],
                                bd8_t[m][:],
                                sq[:, fn * FH:(fn + 1) * FH],
                                start=(m % 4 == 0), stop=(m % 4 == 3))

                def qn_chunk(c, invn_tile, r0):
                    """broadcast invn rows for chunk c and normalize into the
                    fp8 zero-plane qn tile; then retire qkT to w1*qk."""
                    ib = pbc.tile([P, N], f32, tag="ib", name="ib")
                    nc.sync.dma_start(
                        ib[0:HD, :].unsqueeze(1),
                        invn_tile[r0:r0 + 1, :].unsqueeze(1).to_broadcast([1, HD, N]))
                    nc.sync.dma_start(
                        ib[HD:P, :].unsqueeze(1),
                        invn_tile[r0 + 1:r0 + 2, :].unsqueeze(1).to_broadcast([1, HD, N]))
                    nc.gpsimd.tensor_mul(qn_t[c][:, 0:N], qkT_t[c][:], ib[:])
                    if not has_pos:
                        nc.gpsimd.tensor_scalar_mul(
                            qkT_t[c][:], qkT_t[c][:], float(w1))

                def do_head(h):
                    c, half = h // 2, h % 2
                    qn3 = r2(qn_t[c][:])
                    dve_mb = DVE_MB_EVEN if h % 2 == 0 else DVE_MB_ODD
                    E_tiles = []
                    for jj in range(4):
                        Et = pE.tile([P, 2 * N], FP8, tag="E", name=f"E{h}_{jj}")
                        E_tiles.append(Et)
                    for mb in range(NCHUNK):
                        pg = psA.tile([P, N], f32, tag="pg", name="pg")
                        for fn in range(2):
                            nc.tensor.matmul(
                                pg[:, fn * FH:(fn + 1) * FH],
                                qn3[HD * half:HD * half + HD, :, mb * P:(mb + 1) * P],
                                qn3[HD * half:HD * half + HD, :, fn * FH:(fn + 1) * FH],
                                start=True, stop=True, perf_mode=DR)
                        dest = E_tiles[mb // 2][:, (mb % 2) * N:(mb % 2 + 1) * N]
                        if mb in dve_mb:
                            nc.vector.tensor_scalar(
                                out=dest.bitcast(I8), in0=pg[:],
                                scalar1=float(sch_a), scalar2=float(sch_b),
                                op0=MULT, op1=ADD)
                        else:
                            nc.scalar.activation(dest, pg[:], Exp, scale=float(ls8))
                    stage = pstage.tile([HD + 1, N], BF16, tag="stage", name="stage")
                    for fn in range(2):
                        pav = pav_pool.tile([HD + 1, FH], f32, tag="pav", name="pav")
                        for jj in range(4):
                            e3 = r2(E_tiles[jj][:])
                            for xa in (xah_t, xal_t):
                                nc.tensor.matmul(
                                    pav[:],
                                    r2(xa[jj][:])[:, :, h * (HD + 1):(h + 1) * (HD + 1)],
                                    e3[:, :, fn * FH:(fn + 1) * FH],
                                    start=(jj == 0 and xa is xah_t),
                                    stop=(jj == 3 and xa is xal_t),
                                    perf_mode=DR)
                        if fn == 1:
                            nc.vector.tensor_copy(stage[:, fn * FH:(fn + 1) * FH], pav[:])
                        else:
                            nc.scalar.activation(stage[:, fn * FH:(fn + 1) * FH], pav[:], Copy)
                    nc.sync.dma_start(out_t[c][HD * half:HD * half + HD, :], stage[0:HD, :])
                    nc.sync.dma_start(zall_t[h:h + 1, :], stage[HD:HD + 1, :])

                def z_batch(c0, c1):
                    # engine partition base must be aligned -> always row 0
                    h1 = 2 * c1
                    nc.gpsimd.tensor_copy(zallf_t[0:h1, :], zall_t[0:h1, :])
                    nc.vector.reciprocal_approx_fast(
                        rz_t[0:h1, :], zallf_t[0:h1, :])
                    for c in range(c0, c1):
                        zb = pzb.tile([P, N], f32, tag="zb", name="zb")
                        nc.sync.dma_start(
                            zb[0:HD, :].unsqueeze(1),
                            rz_t[2 * c:2 * c + 1, :].unsqueeze(1).to_broadcast([1, HD, N]))
                        nc.sync.dma_start(
                            zb[HD:P, :].unsqueeze(1),
                            rz_t[2 * c + 1:2 * c + 2, :].unsqueeze(1).to_broadcast([1, HD, N]))
                        if c == NCHUNK - 1:
                            nc.vector.tensor_mul(out_t[c][:], out_t[c][:], zb[:])
                        else:
                            nc.gpsimd.tensor_mul(out_t[c][:], out_t[c][:], zb[:])
                        if has_pos:
                            nc.vector.scalar_tensor_tensor(
                                out=out_t[c][:], in0=xbf_t[c][:], scalar=float(w1),
                                in1=out_t[c][:], op0=MULT, op1=ADD)

                # ---- batch A: chunks 0-3, invn for heads 0-7 ----
                with tc.tile_pool(name="psum_sa", bufs=1, space="PSUM") as pApool:
                    ssqA = pApool.tile([8, N], f32, tag="ssqa", name="ssqa")
                    for m in range(4):
                        proj_chunk(m, ssqA)
                    nc.scalar.activation(lns_t[:], ssqA[:], Ln, bias=eps_t[0:8, :],
                                         scale=float(1.0 / (QS * QS)))
                    nc.scalar.activation(invnA_t[:], lns_t[:], Exp, scale=-0.5)
                    for c in range(4):
                        qn_chunk(c, invnA_t, 2 * c)

                # ---- batch B: chunks 4-7, invn for heads 8-15 ----
                with tc.tile_pool(name="psum_sb", bufs=1, space="PSUM") as pBpool:
                    ssqB = pBpool.tile([8, N], f32, tag="ssqb", name="ssqb")
                    for m in range(4, NCHUNK):
                        proj_chunk(m, ssqB)
                    nc.scalar.activation(lns_t[:], ssqB[:], Ln, bias=eps_t[0:8, :],
                                         scale=float(1.0 / (QS * QS)))
                    nc.scalar.activation(invnB_t[:], lns_t[:], Exp, scale=-0.5)
                    for c in range(4, NCHUNK):
                        qn_chunk(c, invnB_t, 2 * (c - 4))

                # ---- heads ----
                with tc.tile_pool(name="psum_av", bufs=2, space="PSUM") as pav_pool_:
                    pav_pool = pav_pool_
                    for h in range(12):
                        do_head(h)
                    z_batch(0, 6)
                    do_head(12)
                    do_head(13)
                    z_batch(6, 7)
                    do_head(14)
                    do_head(15)
                    z_batch(7, 8)

            with (
                tc.tile_pool(name="psum_p2", bufs=8, space="PSUM") as pp2,
                tc.tile_pool(name="fin2", bufs=2) as pfin2,
            ):
                for m in range(NCHUNK):
                    fin = pfin2.tile([P, N], f32, tag="fin", name="fin")
                    for fn in range(2):
                        ps2 = pp2.tile([P, FH], f32, tag="p2", name="p2")
                        for k in range(NCHUNK):
                            nc.tensor.matmul(
                                ps2[:],
                                wt_t[k][:, m * P:(m + 1) * P],
                                out_t[k][:, fn * FH:(fn + 1) * FH],
                                start=(k == 0), stop=(k == NCHUNK - 1))
                        if has_pos:
                            nc.vector.tensor_scalar_add(
                                fin[:, fn * FH:(fn + 1) * FH], ps2[:],
                                bmat_t[:, m:m + 1])
                        else:
                            nc.vector.scalar_tensor_tensor(
                                out=fin[:, fn * FH:(fn + 1) * FH], in0=ps2[:],
                                scalar=bpr_t[:, m:m + 1],
                                in1=qkT_t[m][:, fn * FH:(fn + 1) * FH],
                                op0=ADD, op1=ADD)
                        nc.sync.dma_start(
                            out_d[m * P:(m + 1) * P, fn * FH:(fn + 1) * FH],
                            fin[:, fn * FH:(fn + 1) * FH])

    nc.compile()
    return nc


def _to_dr(a2d):
    """[1024, M] contraction-major -> DoubleRow tiles [4*128, 2*M]."""
    K, M = a2d.shape
    assert K == 1024
    return np.ascontiguousarray(
        a2d.reshape(4, 2, 128, M).transpose(0, 2, 1, 3).reshape(512, 2 * M))


def _host_prep(x, pos, W, b, gamma, w0, w1):
    import ml_dtypes
    F8 = ml_dtypes.float8_e4m3
    BF = ml_dtypes.bfloat16

    WT = np.ascontiguousarray(W.T) * WS               # [C, D], scaled
    wh = WT.astype(F8)
    wl = (WT - wh.astype(np.float32)).astype(F8)
    wdrh = _to_dr(np.asarray(wh))
    wdrl = _to_dr(np.asarray(wl))
    wt_bf = np.ascontiguousarray(W.T).astype(BF)
    bmat = np.ascontiguousarray(b.reshape(NCHUNK, P).T)
    bpr = np.ascontiguousarray((w0 * b).reshape(NCHUNK, P).T)
    bd8 = np.zeros((C, 8), dtype=F8)
    for c in range(NCHUNK):
        col = 2 * (c % 4)
        bd8[c * P:c * P + HD, col] = 1.0
        bd8[c * P + HD:(c + 1) * P, col + 1] = 1.0

    in_maps = []
    for i in range(B):
        xi = x[i]                                     # [N, C]
        if gamma != 0.0:
            xpi = xi + gamma * pos[i].reshape(C, N).T
        else:
            xpi = xi
        xT = np.ascontiguousarray(xpi.T) * XS
        xh8 = xT.astype(F8)
        xl8 = (xT - xh8.astype(np.float32)).astype(F8)
        # augmented v in AV DoubleRow layout: tokens are the contraction
        xaug = np.zeros((N, HEADS * (HD + 1)), dtype=np.float32)
        for h in range(HEADS):
            xaug[:, h * (HD + 1):h * (HD + 1) + HD] = xi[:, h * HD:(h + 1) * HD]
            xaug[:, h * (HD + 1) + HD] = 1.0 / w0
        ah = xaug.astype(F8)
        al = (xaug - ah.astype(np.float32)).astype(F8)
        al[:, (HD + 1) - 1::(HD + 1)] = 0.0           # ones column only in hi
        m = {
            "xdrh": _to_dr(np.asarray(xh8)),
            "xdrl": _to_dr(np.asarray(xl8)),
            "wdrh": wdrh,
            "wdrl": wdrl,
            "xah": _to_dr(np.asarray(ah)),
            "xal": _to_dr(np.asarray(al)),
            "wt": wt_bf,
            "bd8": bd8,
            "bmat": bmat,
            "bpr": bpr,
        }
        if gamma != 0.0:
            m["xbf"] = np.ascontiguousarray(xi.T).astype(BF)
        in_maps.append(m)
    return in_maps


LAST_RESULT = None


def kernel(x, pos, W, b, gamma, attn_gamma, sum_gamma0, sum_gamma1):
    global LAST_RESULT
    import sys
    sys.path.insert(0, "/opt/trn_rl_repo")
    from concourse.bass_utils import run_bass_kernel_spmd

    x = np.asarray(x, dtype=np.float32)
    pos = np.asarray(pos, dtype=np.float32)
    W = np.asarray(W, dtype=np.float32)
    b = np.asarray(b, dtype=np.float32)
    gamma = float(np.asarray(gamma))
    attn_gamma = float(np.asarray(attn_gamma))
    g0 = math.exp(float(np.asarray(sum_gamma0)))
    g1 = math.exp(float(np.asarray(sum_gamma1)))
    w0, w1 = g0 / (g0 + g1), g1 / (g0 + g1)
    logit_scale = math.sqrt(HD) / attn_gamma

    nc = _build(gamma, w0, w1, logit_scale)
    in_maps = _host_prep(x, pos, W, b, gamma, w0, w1)
    res = run_bass_kernel_spmd(
        nc, in_maps, core_ids=list(range(B)),
        trace=os.environ.get("BK_TRACE", "0") == "1",
    )
    LAST_RESULT = res
    out = np.empty((B, N, D), dtype=np.float32)
    for i in range(B):
        out[i] = res.results[i]["out"].T
    return out


# revision 23
# speedup vs baseline: 1.0140x; 1.0140x over previous
"""Trainium2 Bass kernel for nn_Attention_27376121544790.

One batch element per NeuronCore (B=8 -> 8 cores, no collectives).

Math per core (transposed [feature, token] device layout):
  qk   = x @ W.T + b                         [N, D] (kept as qkT [D, N])
  q = k = l2norm(qk per 64-dim head)
  E    = exp((q @ q.T) * sqrt(64)/attn_gamma)  per head, dense softmax numerator
  Z    = column sums of E (ones-column in the augmented v)
  out  = (E.T @ v) / Z
  final= w0*(out @ W.T) + w1*qk + w0*b       since x @ W.T == qk - b (gamma==0)

Speed levers vs the bf16 baseline:
  * fp8e4 DoubleRow matmuls (0.5 cyc/row, 2x contraction per instr).  The
    projection runs 3 hi/lo-split terms (xh@Wh + xl@Wh + xh@Wl) because its
    result feeds the output directly through the w1*qk reuse term; the gram
    uses a zero second plane (contraction is only 64); attn@v splits v hi+lo.
    Host pre-scales x by 8 and W by 64 to dodge fp8 denormals; the
    projection drain descales.
  * exp work is split within each head across ACT (table exp -> fp8) and DVE
    (Schraudolph: i8 = round(g*A + B) bitcast to fp8e4) -- the two engines
    that can read PSUM.  GPSIMD cannot touch PSUM; it gets SBUF-only work
    (qn normalize, out scaling, w1*qk).  Squares for the norms run on ACT
    (Square needs no table load).
  * half-partition broadcasts (invn, w0/Z) via 0-stride-source DMAs instead
    of PE broadcast matmuls, so their consumers can run on GPSIMD.
  * the final blend collapses into the projection drain via the qk reuse
    identity; Z for chunks 0-6 is processed while heads 14/15 still run, so
    the final projection overlaps the attention tail.
"""

import math
import os

import numpy as np

B, N, C, D = 8, 1024, 1024, 1024
HEADS, HD = 16, 64
P = 128
EPS = 1e-6
NCHUNK = C // P   # 8 feature chunks of 128
FH = 512          # free-dim half (PSUM bank width in f32)
XS, WS, QS = 8.0, 64.0, 8.0   # fp8 pre-scales for x, W, qn
LOG2E = 1.0 / math.log(2.0)


def _build(gamma: float, w0: float, w1: float, logit_scale: float):
    import concourse.bass as bass
    import concourse.tile as tile
    from concourse import bacc, mybir

    f32 = mybir.dt.float32
    BF16 = mybir.dt.bfloat16
    FP8 = mybir.dt.float8e4
    I8 = mybir.dt.int8
    DR = mybir.MatmulPerfMode.DoubleRow

    Exp = mybir.ActivationFunctionType.Exp
    Ln = mybir.ActivationFunctionType.Ln
    Square = mybir.ActivationFunctionType.Square
    Copy = mybir.ActivationFunctionType.Copy
    MULT = mybir.AluOpType.mult
    ADD = mybir.AluOpType.add

    # per-head exp-engine split: DVE handles these mb indices (rest ACT)
    DVE_MB_EVEN = frozenset({0, 2, 4, 6})
    DVE_MB_ODD = frozenset({1, 3, 5})
    # stage eviction: heads with h%4 < SDVE4 evict via DVE, rest via ACT
    SDVE4 = int(os.environ.get("BK_SDVE4", "1"))

    has_pos = gamma != 0.0

    ls8 = logit_scale / (QS * QS)
    sch_a = ls8 * 8.0 * LOG2E
    sch_b = 56.0 - 8.0 * 0.0434

    nc = bacc.Bacc("TRN2", target_bir_lowering=False, debug=False)

    xdrh_d = nc.declare_dram_parameter("xdrh", [4 * P, 2 * N], FP8, isOutput=False)
    xdrl_d = nc.declare_dram_parameter("xdrl", [4 * P, 2 * N], FP8, isOutput=False)
    wdrh_d = nc.declare_dram_parameter("wdrh", [4 * P, 2 * D], FP8, isOutput=False)
    wdrl_d = nc.declare_dram_parameter("wdrl", [4 * P, 2 * D], FP8, isOutput=False)
    xah_d = nc.declare_dram_parameter("xah", [4 * P, 2 * HEADS * (HD + 1)], FP8, isOutput=False)
    xal_d = nc.declare_dram_parameter("xal", [4 * P, 2 * HEADS * (HD + 1)], FP8, isOutput=False)
    wt_d = nc.declare_dram_parameter("wt", [C, D], BF16, isOutput=False)
    bd8_d = nc.declare_dram_parameter("bd8", [C, 8], FP8, isOutput=False)
    bmat_d = nc.declare_dram_parameter("bmat", [P, NCHUNK], f32, isOutput=False)
    bpr_d = nc.declare_dram_parameter("bpr", [P, NCHUNK], f32, isOutput=False)
    if has_pos:
        xbf_d = nc.declare_dram_parameter("xbf", [C, N], BF16, isOutput=False)
    out_d = nc.declare_dram_parameter("out", [D, N], f32, isOutput=True)

    AW = HEADS * (HD + 1)  # 1040, xaug free width per plane

    def r2(ap):
        return ap.rearrange("p (two n) -> p two n", two=2)

    with tile.TileContext(nc) as tc:
        with (
            tc.tile_pool(name="persist", bufs=1) as pers,
            tc.tile_pool(name="small", bufs=1) as small,
        ):
            qkT_t = [pers.tile([P, N], f32, tag=f"qk{c}", name=f"qk{c}") for c in range(NCHUNK)]
            qn_t = [pers.tile([P, 2 * N], FP8, tag=f"qn{c}", name=f"qn{c}") for c in range(NCHUNK)]
            out_t = [pers.tile([P, N], BF16, tag=f"ot{c}", name=f"ot{c}") for c in range(NCHUNK)]
            wt_t = [pers.tile([P, D], BF16, tag=f"wt{c}", name=f"wt{c}") for c in range(NCHUNK)]
            xah_t = [pers.tile([P, 2 * AW], FP8, tag=f"xh{j}", name=f"xh{j}") for j in range(4)]
            xal_t = [pers.tile([P, 2 * AW], FP8, tag=f"xl{j}", name=f"xl{j}") for j in range(4)]
            bd8_t = [small.tile([P, 8], FP8, tag=f"bd8{c}", name=f"bd8{c}") for c in range(NCHUNK)]
            if has_pos:
                xbf_t = [pers.tile([P, N], BF16, tag=f"xb{c}", name=f"xb{c}") for c in range(NCHUNK)]
            bmat_t = small.tile([P, NCHUNK], f32, tag="bmat", name="bmat")
            bpr_t = small.tile([P, NCHUNK], f32, tag="bpr", name="bpr")
            invnA_t = small.tile([8, N], f32, tag="invnA", name="invnA")
            invnB_t = small.tile([8, N], f32, tag="invnB", name="invnB")
            lns_t = small.tile([8, N], f32, tag="lns", name="lns")
            zall_t = small.tile([HEADS, N], BF16, tag="zall", name="zall")
            zallf_t = small.tile([HEADS, N], f32, tag="zallf", name="zallf")
            rz_t = small.tile([HEADS, N], f32, tag="rz", name="rz")
            eps_t = small.tile([HEADS, 1], f32, tag="eps", name="eps")
            dummy_t = small.tile([1, 16], f32, tag="dummy", name="dummy")

            # preload the ln/exp ACT table set during input DMA; ln and exp
            # share set 6 so no further table swaps happen
            nc.gpsimd.memset(dummy_t[:], 1.0)
            nc.scalar.activation(dummy_t[:], dummy_t[:], Ln)
            nc.gpsimd.memset(eps_t[:], EPS / (QS * QS))
            for c in range(NCHUNK):
                nc.gpsimd.memset(qn_t[c][:, N:2 * N], 0.0)

            with (
                tc.tile_pool(name="dr_in", bufs=1) as pdr,
                tc.tile_pool(name="psA", bufs=3, space="PSUM") as psA,
                tc.tile_pool(name="sq", bufs=5) as psq,
                tc.tile_pool(name="bcast", bufs=2) as pbc,
                tc.tile_pool(name="E", bufs=8) as pE,
                tc.tile_pool(name="stage", bufs=3) as pstage,
                tc.tile_pool(name="zb", bufs=2) as pzb,
            ):
                wdh_t = [pdr.tile([P, 2 * D], FP8, tag=f"wdh{k}", name=f"wdh{k}") for k in range(4)]
                xdh_t = [pdr.tile([P, 2 * N], FP8, tag=f"xdh{k}", name=f"xdh{k}") for k in range(4)]
                wdl_t = [pdr.tile([P, 2 * D], FP8, tag=f"wdl{k}", name=f"wdl{k}") for k in range(4)]
                xdl_t = [pdr.tile([P, 2 * N], FP8, tag=f"xdl{k}", name=f"xdl{k}") for k in range(4)]
                for k in range(4):
                    nc.sync.dma_start(wdh_t[k][:], wdrh_d[k * P:(k + 1) * P, :])
                    nc.sync.dma_start(xdh_t[k][:], xdrh_d[k * P:(k + 1) * P, :])
                for k in range(4):
                    nc.sync.dma_start(xdl_t[k][:], xdrl_d[k * P:(k + 1) * P, :])
                    nc.sync.dma_start(wdl_t[k][:], wdrl_d[k * P:(k + 1) * P, :])
                for c in range(NCHUNK):
                    nc.sync.dma_start(bd8_t[c][:], bd8_d[c * P:(c + 1) * P, :])
                nc.sync.dma_start(bmat_t[:], bmat_d[:])
                nc.sync.dma_start(bpr_t[:], bpr_d[:])
                for j in range(4):
                    nc.sync.dma_start(xah_t[j][:], xah_d[j * P:(j + 1) * P, :])
                for j in range(4):
                    nc.sync.dma_start(xal_t[j][:], xal_d[j * P:(j + 1) * P, :])
                for c in range(NCHUNK):
                    nc.sync.dma_start(wt_t[c][:], wt_d[c * P:(c + 1) * P, :])
                if has_pos:
                    for c in range(NCHUNK):
                        nc.sync.dma_start(xbf_t[c][:], xbf_d[c * P:(c + 1) * P, :])

                terms = [(wdh_t, xdh_t), (wdh_t, xdl_t), (wdl_t, xdh_t)]
                sq_t = [None] * NCHUNK
                pav_pool = None

                def proj_chunk(m, ssq_ps):
                    """proj1 chunk m (3-term hi/lo DR) + drain + square + its
                    ssq contribution (rows 2*(m%4) of the batch psum)."""
                    ps = psA.tile([P, N], f32, tag="pg", name="pg")
                    for fn in range(2):
                        first = True
                        for wt_dr, xt_dr in terms:
                            for kk in range(4):
                                nc.tensor.matmul(
                                    ps[:, fn * FH:(fn + 1) * FH],
                                    r2(wt_dr[kk][:])[:, :, m * P:(m + 1) * P],
                                    r2(xt_dr[kk][:])[:, :, fn * FH:(fn + 1) * FH],
                                    start=first,
                                    stop=(wt_dr is wdl_t and kk == 3),
                                    perf_mode=DR)
                                first = False
                    nc.vector.tensor_scalar(
                        out=qkT_t[m][:], in0=ps[:], scalar1=1.0 / (XS * WS),
                        scalar2=bmat_t[:, m:m + 1], op0=MULT, op1=ADD)
                    sq = psq.tile([P, N], FP8, tag="sq", name="sq")
                    sq_t[m] = sq
                    nc.scalar.activation(sq[:], qkT_t[m][:], Square)
                    if ssq_ps is not None:
                        for fn in range(2):
                            nc.tensor.matmul(
                                ssq_ps[:, fn * FH:(fn + 1) * FH],
                                bd8_t[m][:],
                                sq[:, fn * FH:(fn + 1) * FH],
                                start=(m % 4 == 0), stop=(m % 4 == 3))

                def qn_chunk(c, invn_tile, r0):
                    """broadcast invn rows for chunk c and normalize into the
                    fp8 zero-plane qn tile; then retire qkT to w1*qk."""
                    ib = pbc.tile([P, N], f32, tag="ib", name="ib")
                    nc.sync.dma_start(
                        ib[0:HD, :].unsqueeze(1),
                        invn_tile[r0:r0 + 1, :].unsqueeze(1).to_broadcast([1, HD, N]))
                    nc.sync.dma_start(
                        ib[HD:P, :].unsqueeze(1),
                        invn_tile[r0 + 1:r0 + 2, :].unsqueeze(1).to_broadcast([1, HD, N]))
                    nc.gpsimd.tensor_mul(qn_t[c][:, 0:N], qkT_t[c][:], ib[:])
                    if not has_pos:
                        nc.gpsimd.tensor_scalar_mul(
                            qkT_t[c][:], qkT_t[c][:], float(w1))

                def do_head(h):
                    c, half = h // 2, h % 2
                    qn3 = r2(qn_t[c][:])
                    dve_mb = DVE_MB_EVEN if h % 2 == 0 else DVE_MB_ODD
                    E_tiles = []
                    for jj in range(4):
                        Et = pE.tile([P, 2 * N], FP8, tag="E", name=f"E{h}_{jj}")
                        E_tiles.append(Et)
                    for mb in range(NCHUNK):
                        pg = psA.tile([P, N], f32, tag="pg", name="pg")
                        for fn in range(2):
                            nc.tensor.matmul(
                                pg[:, fn * FH:(fn + 1) * FH],
                                qn3[HD * half:HD * half + HD, :, mb * P:(mb + 1) * P],
                                qn3[HD * half:HD * half + HD, :, fn * FH:(fn + 1) * FH],
                                start=True, stop=True, perf_mode=DR)
                        dest = E_tiles[mb // 2][:, (mb % 2) * N:(mb % 2 + 1) * N]
                        if mb in dve_mb:
                            nc.vector.tensor_scalar(
                                out=dest.bitcast(I8), in0=pg[:],
                                scalar1=float(sch_a), scalar2=float(sch_b),
                                op0=MULT, op1=ADD)
                        else:
                            nc.scalar.activation(dest, pg[:], Exp, scale=float(ls8))
                    stage = pstage.tile([HD + 1, N], BF16, tag="stage", name="stage")
                    for fn in range(2):
                        pav = pav_pool.tile([HD + 1, FH], f32, tag="pav", name="pav")
                        for jj in range(4):
                            e3 = r2(E_tiles[jj][:])
                            for xa in (xah_t, xal_t):
                                nc.tensor.matmul(
                                    pav[:],
                                    r2(xa[jj][:])[:, :, h * (HD + 1):(h + 1) * (HD + 1)],
                                    e3[:, :, fn * FH:(fn + 1) * FH],
                                    start=(jj == 0 and xa is xah_t),
                                    stop=(jj == 3 and xa is xal_t),
                                    perf_mode=DR)
                        if fn == 1:
                            nc.vector.tensor_copy(stage[:, fn * FH:(fn + 1) * FH], pav[:])
                        else:
                            nc.scalar.activation(stage[:, fn * FH:(fn + 1) * FH], pav[:], Copy)
                    nc.sync.dma_start(out_t[c][HD * half:HD * half + HD, :], stage[0:HD, :])
                    nc.sync.dma_start(zall_t[h:h + 1, :], stage[HD:HD + 1, :])

                def z_batch(c0, c1):
                    # engine partition base must be aligned -> always row 0
                    h1 = 2 * c1
                    nc.gpsimd.tensor_copy(zallf_t[0:h1, :], zall_t[0:h1, :])
                    nc.vector.reciprocal_approx_fast(
                        rz_t[0:h1, :], zallf_t[0:h1, :])
                    for c in range(c0, c1):
                        zb = pzb.tile([P, N], f32, tag="zb", name="zb")
                        nc.sync.dma_start(
                            zb[0:HD, :].unsqueeze(1),
                            rz_t[2 * c:2 * c + 1, :].unsqueeze(1).to_broadcast([1, HD, N]))
                        nc.sync.dma_start(
                            zb[HD:P, :].unsqueeze(1),
                            rz_t[2 * c + 1:2 * c + 2, :].unsqueeze(1).to_broadcast([1, HD, N]))
                        if c == NCHUNK - 1:
                            nc.vector.tensor_mul(out_t[c][:], out_t[c][:], zb[:])
                        else:
                            nc.gpsimd.tensor_mul(out_t[c][:], out_t[c][:], zb[:])
                        if has_pos:
                            nc.vector.scalar_tensor_tensor(
                                out=out_t[c][:], in0=xbf_t[c][:], scalar=float(w1),
                                in1=out_t[c][:], op0=MULT, op1=ADD)

                # ---- batch A: chunks 0-3, invn for heads 0-7 ----
                with tc.tile_pool(name="psum_sa", bufs=1, space="PSUM") as pApool:
                    ssqA = pApool.tile([8, N], f32, tag="ssqa", name="ssqa")
                    for m in range(4):
                        proj_chunk(m, ssqA)
                    nc.scalar.activation(lns_t[:], ssqA[:], Ln, bias=eps_t[0:8, :],
                                         scale=float(1.0 / (QS * QS)))
                    nc.scalar.activation(invnA_t[:], lns_t[:], Exp, scale=-0.5)
                    for c in range(4):
                        qn_chunk(c, invnA_t, 2 * c)

                # ---- batch B: chunks 4-7, invn for heads 8-15 ----
                with tc.tile_pool(name="psum_sb", bufs=1, space="PSUM") as pBpool:
                    ssqB = pBpool.tile([8, N], f32, tag="ssqb", name="ssqb")
                    for m in range(4, NCHUNK):
                        proj_chunk(m, ssqB)
                    nc.scalar.activation(lns_t[:], ssqB[:], Ln, bias=eps_t[0:8, :],
                                         scale=float(1.0 / (QS * QS)))
                    nc.scalar.activation(invnB_t[:], lns_t[:], Exp, scale=-0.5)
                    for c in range(4, NCHUNK):
                        qn_chunk(c, invnB_t, 2 * (c - 4))

                # ---- heads ----
                with tc.tile_pool(name="psum_av", bufs=2, space="PSUM") as pav_pool_:
                    pav_pool = pav_pool_
                    for h in range(12):
                        do_head(h)
                    z_batch(0, 6)
                    do_head(12)
                    do_head(13)
                    z_batch(6, 7)
                    do_head(14)
                    do_head(15)
                    z_batch(7, 8)

            with (
                tc.tile_pool(name="psum_p2", bufs=8, space="PSUM") as pp2,
                tc.tile_pool(name="fin2", bufs=2) as pfin2,
            ):
                for m in range(NCHUNK):
                    fin = pfin2.tile([P, N], f32, tag="fin", name="fin")
                    for fn in range(2):
                        ps2 = pp2.tile([P, FH], f32, tag="p2", name="p2")
                        for k in range(NCHUNK):
                            nc.tensor.matmul(
                                ps2[:],
                                wt_t[k][:, m * P:(m + 1) * P],
                                out_t[k][:, fn * FH:(fn + 1) * FH],
                                start=(k == 0), stop=(k == NCHUNK - 1))
                        if has_pos:
                            nc.vector.tensor_scalar_add(
                                fin[:, fn * FH:(fn + 1) * FH], ps2[:],
                                bmat_t[:, m:m + 1])
                        else:
                            nc.vector.scalar_tensor_tensor(
                                out=fin[:, fn * FH:(fn + 1) * FH], in0=ps2[:],
                                scalar=bpr_t[:, m:m + 1],
                                in1=qkT_t[m][:, fn * FH:(fn + 1) * FH],
                                op0=ADD, op1=ADD)
                        nc.sync.dma_start(
                            out_d[m * P:(m + 1) * P, fn * FH:(fn + 1) * FH],
                            fin[:, fn * FH:(fn + 1) * FH])

    nc.compile()
    return nc


def _to_dr(a2d):
    """[1024, M] contraction-major -> DoubleRow tiles [4*128, 2*M]."""
    K, M = a2d.shape
    assert K == 1024
    return np.ascontiguousarray(
        a2d.reshape(4, 2, 128, M).transpose(0, 2, 1, 3).reshape(512, 2 * M))


def _host_prep(x, pos, W, b, gamma, w0, w1):
    import ml_dtypes
    F8 = ml_dtypes.float8_e4m3
    BF = ml_dtypes.bfloat16

    WT = np.ascontiguousarray(W.T) * WS               # [C, D], scaled
    wh = WT.astype(F8)
    wl = (WT - wh.astype(np.float32)).astype(F8)
    wdrh = _to_dr(np.asarray(wh))
    wdrl = _to_dr(np.asarray(wl))
    wt_bf = np.ascontiguousarray(W.T).astype(BF)
    bmat = np.ascontiguousarray(b.reshape(NCHUNK, P).T)
    bpr = np.ascontiguousarray((w0 * b).reshape(NCHUNK, P).T)
    bd8 = np.zeros((C, 8), dtype=F8)
    for c in range(NCHUNK):
        col = 2 * (c % 4)
        bd8[c * P:c * P + HD, col] = 1.0
        bd8[c * P + HD:(c + 1) * P, col + 1] = 1.0

    in_maps = []
    for i in range(B):
        xi = x[i]                                     # [N, C]
        if gamma != 0.0:
            xpi = xi + gamma * pos[i].reshape(C, N).T
        else:
            xpi = xi
        xT = np.ascontiguousarray(xpi.T) * XS
        xh8 = xT.astype(F8)
        xl8 = (xT - xh8.astype(np.float32)).astype(F8)
        # augmented v in AV DoubleRow layout: tokens are the contraction
        xaug = np.zeros((N, HEADS * (HD + 1)), dtype=np.float32)
        for h in range(HEADS):
            xaug[:, h * (HD + 1):h * (HD + 1) + HD] = xi[:, h * HD:(h + 1) * HD]
            xaug[:, h * (HD + 1) + HD] = 1.0 / w0
        ah = xaug.astype(F8)
        al = (xaug - ah.astype(np.float32)).astype(F8)
        al[:, (HD + 1) - 1::(HD + 1)] = 0.0           # ones column only in hi
        m = {
            "xdrh": _to_dr(np.asarray(xh8)),
            "xdrl": _to_dr(np.asarray(xl8)),
            "wdrh": wdrh,
            "wdrl": wdrl,
            "xah": _to_dr(np.asarray(ah)),
            "xal": _to_dr(np.asarray(al)),
            "wt": wt_bf,
            "bd8": bd8,
            "bmat": bmat,
            "bpr": bpr,
        }
        if gamma != 0.0:
            m["xbf"] = np.ascontiguousarray(xi.T).astype(BF)
        in_maps.append(m)
    return in_maps


LAST_RESULT = None


def kernel(x, pos, W, b, gamma, attn_gamma, sum_gamma0, sum_gamma1):
    global LAST_RESULT
    import sys
    sys.path.insert(0, "/opt/trn_rl_repo")
    from concourse.bass_utils import run_bass_kernel_spmd

    x = np.asarray(x, dtype=np.float32)
    pos = np.asarray(pos, dtype=np.float32)
    W = np.asarray(W, dtype=np.float32)
    b = np.asarray(b, dtype=np.float32)
    gamma = float(np.asarray(gamma))
    attn_gamma = float(np.asarray(attn_gamma))
    g0 = math.exp(float(np.asarray(sum_gamma0)))
    g1 = math.exp(float(np.asarray(sum_gamma1)))
    w0, w1 = g0 / (g0 + g1), g1 / (g0 + g1)
    logit_scale = math.sqrt(HD) / attn_gamma

    nc = _build(gamma, w0, w1, logit_scale)
    in_maps = _host_prep(x, pos, W, b, gamma, w0, w1)
    res = run_bass_kernel_spmd(
        nc, in_maps, core_ids=list(range(B)),
        trace=os.environ.get("BK_TRACE", "0") == "1",
    )
    LAST_RESULT = res
    out = np.empty((B, N, D), dtype=np.float32)
    for i in range(B):
        out[i] = res.results[i]["out"].T
    return out


# revision 24
# speedup vs baseline: 1.0193x; 1.0052x over previous
"""Trainium2 Bass kernel for nn_Attention_27376121544790.

One batch element per NeuronCore (B=8 -> 8 cores, no collectives).

Math per core (transposed [feature, token] device layout):
  qk   = x @ W.T + b                         [N, D] (kept as qkT [D, N])
  q = k = l2norm(qk per 64-dim head)
  E    = exp((q @ q.T) * sqrt(64)/attn_gamma)  per head, dense softmax numerator
  Z    = column sums of E (ones-column in the augmented v)
  out  = (E.T @ v) / Z
  final= w0*(out @ W.T) + w1*qk + w0*b       since x @ W.T == qk - b (gamma==0)

Speed levers vs the bf16 baseline:
  * fp8e4 DoubleRow matmuls (0.5 cyc/row, 2x contraction per instr).  The
    projection runs 3 hi/lo-split terms (xh@Wh + xl@Wh + xh@Wl) because its
    result feeds the output directly through the w1*qk reuse term; the gram
    uses a zero second plane (contraction is only 64); attn@v splits v hi+lo.
    Host pre-scales x by 8 and W by 64 to dodge fp8 denormals; the
    projection drain descales.
  * exp work is split within each head across ACT (table exp -> fp8) and DVE
    (Schraudolph: i8 = round(g*A + B) bitcast to fp8e4) -- the two engines
    that can read PSUM.  GPSIMD cannot touch PSUM; it gets SBUF-only work
    (qn normalize, out scaling, w1*qk).  Squares for the norms run on ACT
    (Square needs no table load).
  * half-partition broadcasts (invn, w0/Z) via 0-stride-source DMAs instead
    of PE broadcast matmuls, so their consumers can run on GPSIMD.
  * the final blend collapses into the projection drain via the qk reuse
    identity; Z for chunks 0-6 is processed while heads 14/15 still run, so
    the final projection overlaps the attention tail.
"""

import math
import os

import numpy as np

B, N, C, D = 8, 1024, 1024, 1024
HEADS, HD = 16, 64
P = 128
EPS = 1e-6
NCHUNK = C // P   # 8 feature chunks of 128
FH = 512          # free-dim half (PSUM bank width in f32)
XS, WS, QS = 8.0, 64.0, 8.0   # fp8 pre-scales for x, W, qn
LOG2E = 1.0 / math.log(2.0)


def _build(gamma: float, w0: float, w1: float, logit_scale: float):
    import concourse.bass as bass
    import concourse.tile as tile
    from concourse import bacc, mybir

    f32 = mybir.dt.float32
    BF16 = mybir.dt.bfloat16
    FP8 = mybir.dt.float8e4
    I8 = mybir.dt.int8
    DR = mybir.MatmulPerfMode.DoubleRow

    Exp = mybir.ActivationFunctionType.Exp
    Ln = mybir.ActivationFunctionType.Ln
    Square = mybir.ActivationFunctionType.Square
    Copy = mybir.ActivationFunctionType.Copy
    MULT = mybir.AluOpType.mult
    ADD = mybir.AluOpType.add

    # per-head exp-engine split: DVE handles these mb indices (rest ACT)
    DVE_MB_EVEN = frozenset({0, 2, 4, 6})
    DVE_MB_ODD = frozenset({1, 3, 5})
    # stage eviction: heads with h%4 < SDVE4 evict via DVE, rest via ACT
    SDVE4 = int(os.environ.get("BK_SDVE4", "1"))

    has_pos = gamma != 0.0

    ls8 = logit_scale / (QS * QS)
    sch_a = ls8 * 8.0 * LOG2E
    sch_b = 56.0 - 8.0 * 0.0434

    nc = bacc.Bacc("TRN2", target_bir_lowering=False, debug=False)

    xdrh_d = nc.declare_dram_parameter("xdrh", [4 * P, 2 * N], FP8, isOutput=False)
    xdrl_d = nc.declare_dram_parameter("xdrl", [4 * P, 2 * N], FP8, isOutput=False)
    wdrh_d = nc.declare_dram_parameter("wdrh", [4 * P, 2 * D], FP8, isOutput=False)
    wdrl_d = nc.declare_dram_parameter("wdrl", [4 * P, 2 * D], FP8, isOutput=False)
    xah_d = nc.declare_dram_parameter("xah", [4 * P, 2 * HEADS * (HD + 1)], FP8, isOutput=False)
    xal_d = nc.declare_dram_parameter("xal", [4 * P, 2 * HEADS * (HD + 1)], FP8, isOutput=False)
    wt_d = nc.declare_dram_parameter("wt", [C, D], BF16, isOutput=False)
    bd8_d = nc.declare_dram_parameter("bd8", [C, 8], FP8, isOutput=False)
    bmat_d = nc.declare_dram_parameter("bmat", [P, NCHUNK], f32, isOutput=False)
    bpr_d = nc.declare_dram_parameter("bpr", [P, NCHUNK], f32, isOutput=False)
    if has_pos:
        xbf_d = nc.declare_dram_parameter("xbf", [C, N], BF16, isOutput=False)
    out_d = nc.declare_dram_parameter("out", [D, N], f32, isOutput=True)

    AW = HEADS * (HD + 1)  # 1040, xaug free width per plane

    def r2(ap):
        return ap.rearrange("p (two n) -> p two n", two=2)

    with tile.TileContext(nc) as tc:
        with (
            tc.tile_pool(name="persist", bufs=1) as pers,
            tc.tile_pool(name="small", bufs=1) as small,
        ):
            qkT_t = [pers.tile([P, N], f32, tag=f"qk{c}", name=f"qk{c}") for c in range(NCHUNK)]
            qn_t = [pers.tile([P, 2 * N], FP8, tag=f"qn{c}", name=f"qn{c}") for c in range(NCHUNK)]
            out_t = [pers.tile([P, N], BF16, tag=f"ot{c}", name=f"ot{c}") for c in range(NCHUNK)]
            wt_t = [pers.tile([P, D], BF16, tag=f"wt{c}", name=f"wt{c}") for c in range(NCHUNK)]
            xah_t = [pers.tile([P, 2 * AW], FP8, tag=f"xh{j}", name=f"xh{j}") for j in range(4)]
            xal_t = [pers.tile([P, 2 * AW], FP8, tag=f"xl{j}", name=f"xl{j}") for j in range(4)]
            bd8_t = [small.tile([P, 8], FP8, tag=f"bd8{c}", name=f"bd8{c}") for c in range(NCHUNK)]
            if has_pos:
                xbf_t = [pers.tile([P, N], BF16, tag=f"xb{c}", name=f"xb{c}") for c in range(NCHUNK)]
            bmat_t = small.tile([P, NCHUNK], f32, tag="bmat", name="bmat")
            bpr_t = small.tile([P, NCHUNK], f32, tag="bpr", name="bpr")
            invnA_t = small.tile([8, N], f32, tag="invnA", name="invnA")
            invnB_t = small.tile([8, N], f32, tag="invnB", name="invnB")
            lns_t = small.tile([8, N], f32, tag="lns", name="lns")
            zall_t = small.tile([HEADS, N], BF16, tag="zall", name="zall")
            zallf_t = small.tile([HEADS, N], f32, tag="zallf", name="zallf")
            rz_t = small.tile([HEADS, N], f32, tag="rz", name="rz")
            eps_t = small.tile([HEADS, 1], f32, tag="eps", name="eps")
            dummy_t = small.tile([1, 16], f32, tag="dummy", name="dummy")

            # preload the ln/exp ACT table set during input DMA; ln and exp
            # share set 6 so no further table swaps happen
            nc.gpsimd.memset(dummy_t[:], 1.0)
            nc.scalar.activation(dummy_t[:], dummy_t[:], Ln)
            nc.gpsimd.memset(eps_t[:], EPS / (QS * QS))
            for c in range(NCHUNK):
                nc.gpsimd.memset(qn_t[c][:, N:2 * N], 0.0)

            with (
                tc.tile_pool(name="dr_in", bufs=1) as pdr,
                tc.tile_pool(name="psA", bufs=3, space="PSUM") as psA,
                tc.tile_pool(name="sq", bufs=5) as psq,
                tc.tile_pool(name="bcast", bufs=2) as pbc,
                tc.tile_pool(name="E", bufs=10) as pE,
                tc.tile_pool(name="stage", bufs=4) as pstage,
                tc.tile_pool(name="zb", bufs=2) as pzb,
            ):
                wdh_t = [pdr.tile([P, 2 * D], FP8, tag=f"wdh{k}", name=f"wdh{k}") for k in range(4)]
                xdh_t = [pdr.tile([P, 2 * N], FP8, tag=f"xdh{k}", name=f"xdh{k}") for k in range(4)]
                wdl_t = [pdr.tile([P, 2 * D], FP8, tag=f"wdl{k}", name=f"wdl{k}") for k in range(4)]
                xdl_t = [pdr.tile([P, 2 * N], FP8, tag=f"xdl{k}", name=f"xdl{k}") for k in range(4)]
                for k in range(4):
                    nc.sync.dma_start(wdh_t[k][:], wdrh_d[k * P:(k + 1) * P, :])
                    nc.sync.dma_start(xdh_t[k][:], xdrh_d[k * P:(k + 1) * P, :])
                for k in range(4):
                    nc.sync.dma_start(xdl_t[k][:], xdrl_d[k * P:(k + 1) * P, :])
                    nc.sync.dma_start(wdl_t[k][:], wdrl_d[k * P:(k + 1) * P, :])
                for c in range(NCHUNK):
                    nc.sync.dma_start(bd8_t[c][:], bd8_d[c * P:(c + 1) * P, :])
                nc.sync.dma_start(bmat_t[:], bmat_d[:])
                nc.sync.dma_start(bpr_t[:], bpr_d[:])
                for j in range(4):
                    nc.sync.dma_start(xah_t[j][:], xah_d[j * P:(j + 1) * P, :])
                for j in range(4):
                    nc.sync.dma_start(xal_t[j][:], xal_d[j * P:(j + 1) * P, :])
                for c in range(NCHUNK):
                    nc.sync.dma_start(wt_t[c][:], wt_d[c * P:(c + 1) * P, :])
                if has_pos:
                    for c in range(NCHUNK):
                        nc.sync.dma_start(xbf_t[c][:], xbf_d[c * P:(c + 1) * P, :])

                terms = [(wdh_t, xdh_t), (wdh_t, xdl_t), (wdl_t, xdh_t)]
                sq_t = [None] * NCHUNK
                pav_pool = None

                def proj_chunk(m, ssq_ps):
                    """proj1 chunk m (3-term hi/lo DR) + drain + square + its
                    ssq contribution (rows 2*(m%4) of the batch psum)."""
                    ps = psA.tile([P, N], f32, tag="pg", name="pg")
                    for fn in range(2):
                        first = True
                        for wt_dr, xt_dr in terms:
                            for kk in range(4):
                                nc.tensor.matmul(
                                    ps[:, fn * FH:(fn + 1) * FH],
                                    r2(wt_dr[kk][:])[:, :, m * P:(m + 1) * P],
                                    r2(xt_dr[kk][:])[:, :, fn * FH:(fn + 1) * FH],
                                    start=first,
                                    stop=(wt_dr is wdl_t and kk == 3),
                                    perf_mode=DR)
                                first = False
                    nc.vector.tensor_scalar(
                        out=qkT_t[m][:], in0=ps[:], scalar1=1.0 / (XS * WS),
                        scalar2=bmat_t[:, m:m + 1], op0=MULT, op1=ADD)
                    sq = psq.tile([P, N], FP8, tag="sq", name="sq")
                    sq_t[m] = sq
                    nc.scalar.activation(sq[:], qkT_t[m][:], Square)
                    if ssq_ps is not None:
                        for fn in range(2):
                            nc.tensor.matmul(
                                ssq_ps[:, fn * FH:(fn + 1) * FH],
                                bd8_t[m][:],
                                sq[:, fn * FH:(fn + 1) * FH],
                                start=(m % 4 == 0), stop=(m % 4 == 3))

                def qn_chunk(c, invn_tile, r0):
                    """broadcast invn rows for chunk c and normalize into the
                    fp8 zero-plane qn tile; then retire qkT to w1*qk."""
                    ib = pbc.tile([P, N], f32, tag="ib", name="ib")
                    nc.sync.dma_start(
                        ib[0:HD, :].unsqueeze(1),
                        invn_tile[r0:r0 + 1, :].unsqueeze(1).to_broadcast([1, HD, N]))
                    nc.sync.dma_start(
                        ib[HD:P, :].unsqueeze(1),
                        invn_tile[r0 + 1:r0 + 2, :].unsqueeze(1).to_broadcast([1, HD, N]))
                    nc.gpsimd.tensor_mul(qn_t[c][:, 0:N], qkT_t[c][:], ib[:])
                    if not has_pos:
                        nc.gpsimd.tensor_scalar_mul(
                            qkT_t[c][:], qkT_t[c][:], float(w1))

                def do_head(h):
                    c, half = h // 2, h % 2
                    qn3 = r2(qn_t[c][:])
                    dve_mb = DVE_MB_EVEN if h % 2 == 0 else DVE_MB_ODD
                    E_tiles = []
                    for jj in range(4):
                        Et = pE.tile([P, 2 * N], FP8, tag="E", name=f"E{h}_{jj}")
                        E_tiles.append(Et)
                    for mb in range(NCHUNK):
                        pg = psA.tile([P, N], f32, tag="pg", name="pg")
                        for fn in range(2):
                            nc.tensor.matmul(
                                pg[:, fn * FH:(fn + 1) * FH],
                                qn3[HD * half:HD * half + HD, :, mb * P:(mb + 1) * P],
                                qn3[HD * half:HD * half + HD, :, fn * FH:(fn + 1) * FH],
                                start=True, stop=True, perf_mode=DR)
                        dest = E_tiles[mb // 2][:, (mb % 2) * N:(mb % 2 + 1) * N]
                        if mb in dve_mb:
                            nc.vector.tensor_scalar(
                                out=dest.bitcast(I8), in0=pg[:],
                                scalar1=float(sch_a), scalar2=float(sch_b),
                                op0=MULT, op1=ADD)
                        else:
                            nc.scalar.activation(dest, pg[:], Exp, scale=float(ls8))
                    stage = pstage.tile([HD + 1, N], BF16, tag="stage", name="stage")
                    for fn in range(2):
                        pav = pav_pool.tile([HD + 1, FH], f32, tag="pav", name="pav")
                        for jj in range(4):
                            e3 = r2(E_tiles[jj][:])
                            for xa in (xah_t, xal_t):
                                nc.tensor.matmul(
                                    pav[:],
                                    r2(xa[jj][:])[:, :, h * (HD + 1):(h + 1) * (HD + 1)],
                                    e3[:, :, fn * FH:(fn + 1) * FH],
                                    start=(jj == 0 and xa is xah_t),
                                    stop=(jj == 3 and xa is xal_t),
                                    perf_mode=DR)
                        if fn == 1:
                            nc.vector.tensor_copy(stage[:, fn * FH:(fn + 1) * FH], pav[:])
                        else:
                            nc.scalar.activation(stage[:, fn * FH:(fn + 1) * FH], pav[:], Copy)
                    nc.sync.dma_start(out_t[c][HD * half:HD * half + HD, :], stage[0:HD, :])
                    nc.sync.dma_start(zall_t[h:h + 1, :], stage[HD:HD + 1, :])

                def z_batch(c0, c1):
                    # engine partition base must be aligned -> always row 0
                    h1 = 2 * c1
                    nc.gpsimd.tensor_copy(zallf_t[0:h1, :], zall_t[0:h1, :])
                    nc.vector.reciprocal_approx_fast(
                        rz_t[0:h1, :], zallf_t[0:h1, :])
                    for c in range(c0, c1):
                        zb = pzb.tile([P, N], f32, tag="zb", name="zb")
                        nc.sync.dma_start(
                            zb[0:HD, :].unsqueeze(1),
                            rz_t[2 * c:2 * c + 1, :].unsqueeze(1).to_broadcast([1, HD, N]))
                        nc.sync.dma_start(
                            zb[HD:P, :].unsqueeze(1),
                            rz_t[2 * c + 1:2 * c + 2, :].unsqueeze(1).to_broadcast([1, HD, N]))
                        if c == NCHUNK - 1:
                            nc.vector.tensor_mul(out_t[c][:], out_t[c][:], zb[:])
                        else:
                            nc.gpsimd.tensor_mul(out_t[c][:], out_t[c][:], zb[:])
                        if has_pos:
                            nc.vector.scalar_tensor_tensor(
                                out=out_t[c][:], in0=xbf_t[c][:], scalar=float(w1),
                                in1=out_t[c][:], op0=MULT, op1=ADD)

                # ---- batch A: chunks 0-3, invn for heads 0-7 ----
                with tc.tile_pool(name="psum_sa", bufs=1, space="PSUM") as pApool:
                    ssqA = pApool.tile([8, N], f32, tag="ssqa", name="ssqa")
                    for m in range(4):
                        proj_chunk(m, ssqA)
                    nc.scalar.activation(lns_t[:], ssqA[:], Ln, bias=eps_t[0:8, :],
                                         scale=float(1.0 / (QS * QS)))
                    nc.scalar.activation(invnA_t[:], lns_t[:], Exp, scale=-0.5)
                    for c in range(4):
                        qn_chunk(c, invnA_t, 2 * c)

                # ---- batch B: chunks 4-7, invn for heads 8-15 ----
                with tc.tile_pool(name="psum_sb", bufs=1, space="PSUM") as pBpool:
                    ssqB = pBpool.tile([8, N], f32, tag="ssqb", name="ssqb")
                    for m in range(4, NCHUNK):
                        proj_chunk(m, ssqB)
                    nc.scalar.activation(lns_t[:], ssqB[:], Ln, bias=eps_t[0:8, :],
                                         scale=float(1.0 / (QS * QS)))
                    nc.scalar.activation(invnB_t[:], lns_t[:], Exp, scale=-0.5)
                    for c in range(4, NCHUNK):
                        qn_chunk(c, invnB_t, 2 * (c - 4))

                # ---- heads ----
                with tc.tile_pool(name="psum_av", bufs=2, space="PSUM") as pav_pool_:
                    pav_pool = pav_pool_
                    for h in range(12):
                        do_head(h)
                    z_batch(0, 6)
                    do_head(12)
                    do_head(13)
                    z_batch(6, 7)
                    do_head(14)
                    do_head(15)
                    z_batch(7, 8)

            with (
                tc.tile_pool(name="psum_p2", bufs=8, space="PSUM") as pp2,
                tc.tile_pool(name="fin2", bufs=2) as pfin2,
            ):
                for m in range(NCHUNK):
                    fin = pfin2.tile([P, N], f32, tag="fin", name="fin")
                    for fn in range(2):
                        ps2 = pp2.tile([P, FH], f32, tag="p2", name="p2")
                        for k in range(NCHUNK):
                            nc.tensor.matmul(
                                ps2[:],
                                wt_t[k][:, m * P:(m + 1) * P],
                                out_t[k][:, fn * FH:(fn + 1) * FH],
                                start=(k == 0), stop=(k == NCHUNK - 1))
                        if has_pos:
                            nc.vector.tensor_scalar_add(
                                fin[:, fn * FH:(fn + 1) * FH], ps2[:],
                                bmat_t[:, m:m + 1])
                        else:
                            nc.vector.scalar_tensor_tensor(
                                out=fin[:, fn * FH:(fn + 1) * FH], in0=ps2[:],
                                scalar=bpr_t[:, m:m + 1],
                                in1=qkT_t[m][:, fn * FH:(fn + 1) * FH],
                                op0=ADD, op1=ADD)
                        nc.sync.dma_start(
                            out_d[m * P:(m + 1) * P, fn * FH:(fn + 1) * FH],
                            fin[:, fn * FH:(fn + 1) * FH])

    nc.compile()
    return nc


def _to_dr(a2d):
    """[1024, M] contraction-major -> DoubleRow tiles [4*128, 2*M]."""
    K, M = a2d.shape
    assert K == 1024
    return np.ascontiguousarray(
        a2d.reshape(4, 2, 128, M).transpose(0, 2, 1, 3).reshape(512, 2 * M))


def _host_prep(x, pos, W, b, gamma, w0, w1):
    import ml_dtypes
    F8 = ml_dtypes.float8_e4m3
    BF = ml_dtypes.bfloat16

    WT = np.ascontiguousarray(W.T) * WS               # [C, D], scaled
    wh = WT.astype(F8)
    wl = (WT - wh.astype(np.float32)).astype(F8)
    wdrh = _to_dr(np.asarray(wh))
    wdrl = _to_dr(np.asarray(wl))
    wt_bf = np.ascontiguousarray(W.T).astype(BF)
    bmat = np.ascontiguousarray(b.reshape(NCHUNK, P).T)
    bpr = np.ascontiguousarray((w0 * b).reshape(NCHUNK, P).T)
    bd8 = np.zeros((C, 8), dtype=F8)
    for c in range(NCHUNK):
        col = 2 * (c % 4)
        bd8[c * P:c * P + HD, col] = 1.0
        bd8[c * P + HD:(c + 1) * P, col + 1] = 1.0

    in_maps = []
    for i in range(B):
        xi = x[i]                                     # [N, C]
        if gamma != 0.0:
            xpi = xi + gamma * pos[i].reshape(C, N).T
        else:
            xpi = xi
        xT = np.ascontiguousarray(xpi.T) * XS
        xh8 = xT.astype(F8)
        xl8 = (xT - xh8.astype(np.float32)).astype(F8)
        # augmented v in AV DoubleRow layout: tokens are the contraction
        xaug = np.zeros((N, HEADS * (HD + 1)), dtype=np.float32)
        for h in range(HEADS):
            xaug[:, h * (HD + 1):h * (HD + 1) + HD] = xi[:, h * HD:(h + 1) * HD]
            xaug[:, h * (HD + 1) + HD] = 1.0 / w0
        ah = xaug.astype(F8)
        al = (xaug - ah.astype(np.float32)).astype(F8)
        al[:, (HD + 1) - 1::(HD + 1)] = 0.0           # ones column only in hi
        m = {
            "xdrh": _to_dr(np.asarray(xh8)),
            "xdrl": _to_dr(np.asarray(xl8)),
            "wdrh": wdrh,
            "wdrl": wdrl,
            "xah": _to_dr(np.asarray(ah)),
            "xal": _to_dr(np.asarray(al)),
            "wt": wt_bf,
            "bd8": bd8,
            "bmat": bmat,
            "bpr": bpr,
        }
        if gamma != 0.0:
            m["xbf"] = np.ascontiguousarray(xi.T).astype(BF)
        in_maps.append(m)
    return in_maps


LAST_RESULT = None


def kernel(x, pos, W, b, gamma, attn_gamma, sum_gamma0, sum_gamma1):
    global LAST_RESULT
    import sys
    sys.path.insert(0, "/opt/trn_rl_repo")
    from concourse.bass_utils import run_bass_kernel_spmd

    x = np.asarray(x, dtype=np.float32)
    pos = np.asarray(pos, dtype=np.float32)
    W = np.asarray(W, dtype=np.float32)
    b = np.asarray(b, dtype=np.float32)
    gamma = float(np.asarray(gamma))
    attn_gamma = float(np.asarray(attn_gamma))
    g0 = math.exp(float(np.asarray(sum_gamma0)))
    g1 = math.exp(float(np.asarray(sum_gamma1)))
    w0, w1 = g0 / (g0 + g1), g1 / (g0 + g1)
    logit_scale = math.sqrt(HD) / attn_gamma

    nc = _build(gamma, w0, w1, logit_scale)
    in_maps = _host_prep(x, pos, W, b, gamma, w0, w1)
    res = run_bass_kernel_spmd(
        nc, in_maps, core_ids=list(range(B)),
        trace=os.environ.get("BK_TRACE", "0") == "1",
    )
    LAST_RESULT = res
    out = np.empty((B, N, D), dtype=np.float32)
    for i in range(B):
        out[i] = res.results[i]["out"].T
    return out


# revision 25
# speedup vs baseline: 1.0379x; 1.0183x over previous
"""Trainium2 Bass kernel for nn_Attention_27376121544790.

One batch element per NeuronCore (B=8 -> 8 cores, no collectives).

Math per core (transposed [feature, token] device layout):
  qk   = x @ W.T + b                         [N, D] (kept as qkT [D, N])
  q = k = l2norm(qk per 64-dim head)
  E    = exp((q @ q.T) * sqrt(64)/attn_gamma)  per head, dense softmax numerator
  Z    = column sums of E (ones-column in the augmented v)
  out  = (E.T @ v) / Z
  final= w0*(out @ W.T) + w1*qk + w0*b       since x @ W.T == qk - b (gamma==0)

Speed levers vs the bf16 baseline:
  * fp8e4 DoubleRow matmuls (0.5 cyc/row, 2x contraction per instr).  The
    projection runs 3 hi/lo-split terms (xh@Wh + xl@Wh + xh@Wl) because its
    result feeds the output directly through the w1*qk reuse term; the gram
    uses a zero second plane (contraction is only 64); attn@v splits v hi+lo.
    Host pre-scales x by 8 and W by 64 to dodge fp8 denormals; the
    projection drain descales.
  * exp work is split within each head across ACT (table exp -> fp8) and DVE
    (Schraudolph: i8 = round(g*A + B) bitcast to fp8e4) -- the two engines
    that can read PSUM.  GPSIMD cannot touch PSUM; it gets SBUF-only work
    (qn normalize, out scaling, w1*qk).  Squares for the norms run on ACT
    (Square needs no table load).
  * half-partition broadcasts (invn, w0/Z) via 0-stride-source DMAs instead
    of PE broadcast matmuls, so their consumers can run on GPSIMD.
  * the final blend collapses into the projection drain via the qk reuse
    identity; Z for chunks 0-6 is processed while heads 14/15 still run, so
    the final projection overlaps the attention tail.
"""

import math
import os

import numpy as np

B, N, C, D = 8, 1024, 1024, 1024
HEADS, HD = 16, 64
P = 128
EPS = 1e-6
NCHUNK = C // P   # 8 feature chunks of 128
FH = 512          # free-dim half (PSUM bank width in f32)
XS, WS, QS = 8.0, 64.0, 8.0   # fp8 pre-scales for x, W, qn
LOG2E = 1.0 / math.log(2.0)


def _build(gamma: float, w0: float, w1: float, logit_scale: float):
    import concourse.bass as bass
    import concourse.tile as tile
    from concourse import bacc, mybir

    f32 = mybir.dt.float32
    BF16 = mybir.dt.bfloat16
    FP8 = mybir.dt.float8e4
    I8 = mybir.dt.int8
    DR = mybir.MatmulPerfMode.DoubleRow

    Exp = mybir.ActivationFunctionType.Exp
    Ln = mybir.ActivationFunctionType.Ln
    Square = mybir.ActivationFunctionType.Square
    Copy = mybir.ActivationFunctionType.Copy
    MULT = mybir.AluOpType.mult
    ADD = mybir.AluOpType.add

    # per-head exp-engine split: DVE handles these mb indices (rest ACT)
    DVE_MB_EVEN = frozenset({0, 2, 4, 6})
    DVE_MB_ODD = frozenset({1, 3, 5})
    # stage eviction: heads with h%4 < SDVE4 evict via DVE, rest via ACT
    SDVE4 = int(os.environ.get("BK_SDVE4", "1"))

    has_pos = gamma != 0.0

    ls8 = logit_scale / (QS * QS)
    sch_a = ls8 * 8.0 * LOG2E
    sch_b = 56.0 - 8.0 * 0.0434

    nc = bacc.Bacc("TRN2", target_bir_lowering=False, debug=False)

    xdrh_d = nc.declare_dram_parameter("xdrh", [4 * P, 2 * N], FP8, isOutput=False)
    xdrl_d = nc.declare_dram_parameter("xdrl", [4 * P, 2 * N], FP8, isOutput=False)
    wdrh_d = nc.declare_dram_parameter("wdrh", [4 * P, 2 * D], FP8, isOutput=False)
    wdrl_d = nc.declare_dram_parameter("wdrl", [4 * P, 2 * D], FP8, isOutput=False)
    xah_d = nc.declare_dram_parameter("xah", [4 * P, 2 * HEADS * (HD + 1)], FP8, isOutput=False)
    xal_d = nc.declare_dram_parameter("xal", [4 * P, 2 * HEADS * (HD + 1)], FP8, isOutput=False)
    wt_d = nc.declare_dram_parameter("wt", [C, D], BF16, isOutput=False)
    bd8_d = nc.declare_dram_parameter("bd8", [C, 8], FP8, isOutput=False)
    bmat_d = nc.declare_dram_parameter("bmat", [P, NCHUNK], f32, isOutput=False)
    bpr_d = nc.declare_dram_parameter("bpr", [P, NCHUNK], f32, isOutput=False)
    if has_pos:
        xbf_d = nc.declare_dram_parameter("xbf", [C, N], BF16, isOutput=False)
    out_d = nc.declare_dram_parameter("out", [D, N], f32, isOutput=True)

    AW = HEADS * (HD + 1)  # 1040, xaug free width per plane

    def r2(ap):
        return ap.rearrange("p (two n) -> p two n", two=2)

    with tile.TileContext(nc) as tc:
        with (
            tc.tile_pool(name="persist", bufs=1) as pers,
            tc.tile_pool(name="small", bufs=1) as small,
        ):
            qkT_t = [pers.tile([P, N], f32, tag=f"qk{c}", name=f"qk{c}") for c in range(NCHUNK)]
            qn_t = [pers.tile([P, 2 * N], FP8, tag=f"qn{c}", name=f"qn{c}") for c in range(NCHUNK)]
            out_t = [pers.tile([P, N], BF16, tag=f"ot{c}", name=f"ot{c}") for c in range(NCHUNK)]
            wt_t = [pers.tile([P, D], BF16, tag=f"wt{c}", name=f"wt{c}") for c in range(NCHUNK)]
            xah_t = [pers.tile([P, 2 * AW], FP8, tag=f"xh{j}", name=f"xh{j}") for j in range(4)]
            xal_t = [pers.tile([P, 2 * AW], FP8, tag=f"xl{j}", name=f"xl{j}") for j in range(4)]
            bd8_t = [small.tile([P, 8], FP8, tag=f"bd8{c}", name=f"bd8{c}") for c in range(NCHUNK)]
            if has_pos:
                xbf_t = [pers.tile([P, N], BF16, tag=f"xb{c}", name=f"xb{c}") for c in range(NCHUNK)]
            bmat_t = small.tile([P, NCHUNK], f32, tag="bmat", name="bmat")
            bpr_t = small.tile([P, NCHUNK], f32, tag="bpr", name="bpr")
            invnA_t = small.tile([8, N], f32, tag="invnA", name="invnA")
            invnB_t = small.tile([8, N], f32, tag="invnB", name="invnB")
            lns_t = small.tile([8, N], f32, tag="lns", name="lns")
            zall_t = small.tile([HEADS, N], BF16, tag="zall", name="zall")
            zallf_t = small.tile([HEADS, N], f32, tag="zallf", name="zallf")
            rz_t = small.tile([HEADS, N], f32, tag="rz", name="rz")
            eps_t = small.tile([HEADS, 1], f32, tag="eps", name="eps")
            dummy_t = small.tile([1, 16], f32, tag="dummy", name="dummy")

            # preload the ln/exp ACT table set during input DMA; ln and exp
            # share set 6 so no further table swaps happen
            nc.gpsimd.memset(dummy_t[:], 1.0)
            nc.scalar.activation(dummy_t[:], dummy_t[:], Ln)
            nc.gpsimd.memset(eps_t[:], EPS / (QS * QS))
            for c in range(NCHUNK):
                nc.gpsimd.memset(qn_t[c][:, N:2 * N], 0.0)

            with (
                tc.tile_pool(name="dr_in", bufs=1) as pdr,
                tc.tile_pool(name="psA", bufs=3, space="PSUM") as psA,
                tc.tile_pool(name="sq", bufs=5) as psq,
                tc.tile_pool(name="bcast", bufs=2) as pbc,
                tc.tile_pool(name="E", bufs=10) as pE,
                tc.tile_pool(name="stage", bufs=4) as pstage,
                tc.tile_pool(name="zb", bufs=3) as pzb,
            ):
                wdh_t = [pdr.tile([P, 2 * D], FP8, tag=f"wdh{k}", name=f"wdh{k}") for k in range(4)]
                xdh_t = [pdr.tile([P, 2 * N], FP8, tag=f"xdh{k}", name=f"xdh{k}") for k in range(4)]
                wdl_t = [pdr.tile([P, 2 * D], FP8, tag=f"wdl{k}", name=f"wdl{k}") for k in range(4)]
                xdl_t = [pdr.tile([P, 2 * N], FP8, tag=f"xdl{k}", name=f"xdl{k}") for k in range(4)]
                for k in range(4):
                    nc.sync.dma_start(wdh_t[k][:], wdrh_d[k * P:(k + 1) * P, :])
                    nc.sync.dma_start(xdh_t[k][:], xdrh_d[k * P:(k + 1) * P, :])
                for k in range(4):
                    nc.sync.dma_start(xdl_t[k][:], xdrl_d[k * P:(k + 1) * P, :])
                    nc.sync.dma_start(wdl_t[k][:], wdrl_d[k * P:(k + 1) * P, :])
                for c in range(NCHUNK):
                    nc.sync.dma_start(bd8_t[c][:], bd8_d[c * P:(c + 1) * P, :])
                nc.sync.dma_start(bmat_t[:], bmat_d[:])
                nc.sync.dma_start(bpr_t[:], bpr_d[:])
                for j in range(4):
                    nc.sync.dma_start(xah_t[j][:], xah_d[j * P:(j + 1) * P, :])
                for j in range(4):
                    nc.sync.dma_start(xal_t[j][:], xal_d[j * P:(j + 1) * P, :])
                for c in range(NCHUNK):
                    nc.sync.dma_start(wt_t[c][:], wt_d[c * P:(c + 1) * P, :])
                if has_pos:
                    for c in range(NCHUNK):
                        nc.sync.dma_start(xbf_t[c][:], xbf_d[c * P:(c + 1) * P, :])

                terms = [(wdh_t, xdh_t), (wdh_t, xdl_t), (wdl_t, xdh_t)]
                sq_t = [None] * NCHUNK
                pav_pool = None

                def proj_chunk(m, ssq_ps):
                    """proj1 chunk m (3-term hi/lo DR) + drain + square + its
                    ssq contribution (rows 2*(m%4) of the batch psum)."""
                    ps = psA.tile([P, N], f32, tag="pg", name="pg")
                    for fn in range(2):
                        first = True
                        for wt_dr, xt_dr in terms:
                            for kk in range(4):
                                nc.tensor.matmul(
                                    ps[:, fn * FH:(fn + 1) * FH],
                                    r2(wt_dr[kk][:])[:, :, m * P:(m + 1) * P],
                                    r2(xt_dr[kk][:])[:, :, fn * FH:(fn + 1) * FH],
                                    start=first,
                                    stop=(wt_dr is wdl_t and kk == 3),
                                    perf_mode=DR)
                                first = False
                    nc.vector.tensor_scalar(
                        out=qkT_t[m][:], in0=ps[:], scalar1=1.0 / (XS * WS),
                        scalar2=bmat_t[:, m:m + 1], op0=MULT, op1=ADD)
                    sq = psq.tile([P, N], FP8, tag="sq", name="sq")
                    sq_t[m] = sq
                    nc.scalar.activation(sq[:], qkT_t[m][:], Square)
                    if ssq_ps is not None:
                        for fn in range(2):
                            nc.tensor.matmul(
                                ssq_ps[:, fn * FH:(fn + 1) * FH],
                                bd8_t[m][:],
                                sq[:, fn * FH:(fn + 1) * FH],
                                start=(m % 4 == 0), stop=(m % 4 == 3))

                def qn_chunk(c, invn_tile, r0):
                    """broadcast invn rows for chunk c and normalize into the
                    fp8 zero-plane qn tile; then retire qkT to w1*qk."""
                    ib = pbc.tile([P, N], f32, tag="ib", name="ib")
                    nc.sync.dma_start(
                        ib[0:HD, :].unsqueeze(1),
                        invn_tile[r0:r0 + 1, :].unsqueeze(1).to_broadcast([1, HD, N]))
                    nc.sync.dma_start(
                        ib[HD:P, :].unsqueeze(1),
                        invn_tile[r0 + 1:r0 + 2, :].unsqueeze(1).to_broadcast([1, HD, N]))
                    nc.gpsimd.tensor_mul(qn_t[c][:, 0:N], qkT_t[c][:], ib[:])
                    if not has_pos:
                        nc.gpsimd.tensor_scalar_mul(
                            qkT_t[c][:], qkT_t[c][:], float(w1))

                def do_head(h):
                    c, half = h // 2, h % 2
                    qn3 = r2(qn_t[c][:])
                    dve_mb = DVE_MB_EVEN if h % 2 == 0 else DVE_MB_ODD
                    E_tiles = []
                    for jj in range(4):
                        Et = pE.tile([P, 2 * N], FP8, tag="E", name=f"E{h}_{jj}")
                        E_tiles.append(Et)
                    for mb in range(NCHUNK):
                        pg = psA.tile([P, N], f32, tag="pg", name="pg")
                        for fn in range(2):
                            nc.tensor.matmul(
                                pg[:, fn * FH:(fn + 1) * FH],
                                qn3[HD * half:HD * half + HD, :, mb * P:(mb + 1) * P],
                                qn3[HD * half:HD * half + HD, :, fn * FH:(fn + 1) * FH],
                                start=True, stop=True, perf_mode=DR)
                        dest = E_tiles[mb // 2][:, (mb % 2) * N:(mb % 2 + 1) * N]
                        if mb in dve_mb:
                            nc.vector.tensor_scalar(
                                out=dest.bitcast(I8), in0=pg[:],
                                scalar1=float(sch_a), scalar2=float(sch_b),
                                op0=MULT, op1=ADD)
                        else:
                            nc.scalar.activation(dest, pg[:], Exp, scale=float(ls8))
                    stage = pstage.tile([HD + 1, N], BF16, tag="stage", name="stage")
                    for fn in range(2):
                        pav = pav_pool.tile([HD + 1, FH], f32, tag="pav", name="pav")
                        for jj in range(4):
                            e3 = r2(E_tiles[jj][:])
                            for xa in (xah_t, xal_t):
                                nc.tensor.matmul(
                                    pav[:],
                                    r2(xa[jj][:])[:, :, h * (HD + 1):(h + 1) * (HD + 1)],
                                    e3[:, :, fn * FH:(fn + 1) * FH],
                                    start=(jj == 0 and xa is xah_t),
                                    stop=(jj == 3 and xa is xal_t),
                                    perf_mode=DR)
                        if fn == 1:
                            nc.vector.tensor_copy(stage[:, fn * FH:(fn + 1) * FH], pav[:])
                        else:
                            nc.scalar.activation(stage[:, fn * FH:(fn + 1) * FH], pav[:], Copy)
                    nc.sync.dma_start(out_t[c][HD * half:HD * half + HD, :], stage[0:HD, :])
                    nc.sync.dma_start(zall_t[h:h + 1, :], stage[HD:HD + 1, :])

                def z_batch(c0, c1):
                    # engine partition base must be aligned -> always row 0
                    h1 = 2 * c1
                    nc.gpsimd.tensor_copy(zallf_t[0:h1, :], zall_t[0:h1, :])
                    nc.vector.reciprocal_approx_fast(
                        rz_t[0:h1, :], zallf_t[0:h1, :])
                    for c in range(c0, c1):
                        zb = pzb.tile([P, N], f32, tag="zb", name="zb")
                        nc.sync.dma_start(
                            zb[0:HD, :].unsqueeze(1),
                            rz_t[2 * c:2 * c + 1, :].unsqueeze(1).to_broadcast([1, HD, N]))
                        nc.sync.dma_start(
                            zb[HD:P, :].unsqueeze(1),
                            rz_t[2 * c + 1:2 * c + 2, :].unsqueeze(1).to_broadcast([1, HD, N]))
                        if c == NCHUNK - 1:
                            nc.vector.tensor_mul(out_t[c][:], out_t[c][:], zb[:])
                        else:
                            nc.gpsimd.tensor_mul(out_t[c][:], out_t[c][:], zb[:])
                        if has_pos:
                            nc.vector.scalar_tensor_tensor(
                                out=out_t[c][:], in0=xbf_t[c][:], scalar=float(w1),
                                in1=out_t[c][:], op0=MULT, op1=ADD)

                # ---- batch A: chunks 0-3, invn for heads 0-7 ----
                with tc.tile_pool(name="psum_sa", bufs=1, space="PSUM") as pApool:
                    ssqA = pApool.tile([8, N], f32, tag="ssqa", name="ssqa")
                    for m in range(4):
                        proj_chunk(m, ssqA)
                    nc.scalar.activation(lns_t[:], ssqA[:], Ln, bias=eps_t[0:8, :],
                                         scale=float(1.0 / (QS * QS)))
                    nc.scalar.activation(invnA_t[:], lns_t[:], Exp, scale=-0.5)
                    for c in range(4):
                        qn_chunk(c, invnA_t, 2 * c)

                # ---- batch B: chunks 4-7, invn for heads 8-15 ----
                with tc.tile_pool(name="psum_sb", bufs=1, space="PSUM") as pBpool:
                    ssqB = pBpool.tile([8, N], f32, tag="ssqb", name="ssqb")
                    for m in range(4, NCHUNK):
                        proj_chunk(m, ssqB)
                    nc.scalar.activation(lns_t[:], ssqB[:], Ln, bias=eps_t[0:8, :],
                                         scale=float(1.0 / (QS * QS)))
                    nc.scalar.activation(invnB_t[:], lns_t[:], Exp, scale=-0.5)
                    for c in range(4, NCHUNK):
                        qn_chunk(c, invnB_t, 2 * (c - 4))

                # ---- heads ----
                with tc.tile_pool(name="psum_av", bufs=2, space="PSUM") as pav_pool_:
                    pav_pool = pav_pool_
                    for h in range(8):
                        do_head(h)
                    z_batch(0, 4)
                    do_head(8)
                    do_head(9)
                    do_head(10)
                    do_head(11)
                    z_batch(4, 6)
                    do_head(12)
                    do_head(13)
                    z_batch(6, 7)
                    do_head(14)
                    do_head(15)
                    z_batch(7, 8)

            with (
                tc.tile_pool(name="psum_p2", bufs=8, space="PSUM") as pp2,
                tc.tile_pool(name="fin2", bufs=2) as pfin2,
            ):
                for m in range(NCHUNK):
                    fin = pfin2.tile([P, N], f32, tag="fin", name="fin")
                    for fn in range(2):
                        ps2 = pp2.tile([P, FH], f32, tag="p2", name="p2")
                        for k in range(NCHUNK):
                            nc.tensor.matmul(
                                ps2[:],
                                wt_t[k][:, m * P:(m + 1) * P],
                                out_t[k][:, fn * FH:(fn + 1) * FH],
                                start=(k == 0), stop=(k == NCHUNK - 1))
                        if has_pos:
                            nc.vector.tensor_scalar_add(
                                fin[:, fn * FH:(fn + 1) * FH], ps2[:],
                                bmat_t[:, m:m + 1])
                        else:
                            nc.vector.scalar_tensor_tensor(
                                out=fin[:, fn * FH:(fn + 1) * FH], in0=ps2[:],
                                scalar=bpr_t[:, m:m + 1],
                                in1=qkT_t[m][:, fn * FH:(fn + 1) * FH],
                                op0=ADD, op1=ADD)
                        nc.sync.dma_start(
                            out_d[m * P:(m + 1) * P, fn * FH:(fn + 1) * FH],
                            fin[:, fn * FH:(fn + 1) * FH])

    nc.compile()
    return nc


def _to_dr(a2d):
    """[1024, M] contraction-major -> DoubleRow tiles [4*128, 2*M]."""
    K, M = a2d.shape
    assert K == 1024
    return np.ascontiguousarray(
        a2d.reshape(4, 2, 128, M).transpose(0, 2, 1, 3).reshape(512, 2 * M))


def _host_prep(x, pos, W, b, gamma, w0, w1):
    import ml_dtypes
    F8 = ml_dtypes.float8_e4m3
    BF = ml_dtypes.bfloat16

    WT = np.ascontiguousarray(W.T) * WS               # [C, D], scaled
    wh = WT.astype(F8)
    wl = (WT - wh.astype(np.float32)).astype(F8)
    wdrh = _to_dr(np.asarray(wh))
    wdrl = _to_dr(np.asarray(wl))
    wt_bf = np.ascontiguousarray(W.T).astype(BF)
    bmat = np.ascontiguousarray(b.reshape(NCHUNK, P).T)
    bpr = np.ascontiguousarray((w0 * b).reshape(NCHUNK, P).T)
    bd8 = np.zeros((C, 8), dtype=F8)
    for c in range(NCHUNK):
        col = 2 * (c % 4)
        bd8[c * P:c * P + HD, col] = 1.0
        bd8[c * P + HD:(c + 1) * P, col + 1] = 1.0

    in_maps = []
    for i in range(B):
        xi = x[i]                                     # [N, C]
        if gamma != 0.0:
            xpi = xi + gamma * pos[i].reshape(C, N).T
        else:
            xpi = xi
        xT = np.ascontiguousarray(xpi.T) * XS
        xh8 = xT.astype(F8)
        xl8 = (xT - xh8.astype(np.float32)).astype(F8)
        # augmented v in AV DoubleRow layout: tokens are the contraction
        xaug = np.zeros((N, HEADS * (HD + 1)), dtype=np.float32)
        for h in range(HEADS):
            xaug[:, h * (HD + 1):h * (HD + 1) + HD] = xi[:, h * HD:(h + 1) * HD]
            xaug[:, h * (HD + 1) + HD] = 1.0 / w0
        ah = xaug.astype(F8)
        al = (xaug - ah.astype(np.float32)).astype(F8)
        al[:, (HD + 1) - 1::(HD + 1)] = 0.0           # ones column only in hi
        m = {
            "xdrh": _to_dr(np.asarray(xh8)),
            "xdrl": _to_dr(np.asarray(xl8)),
            "wdrh": wdrh,
            "wdrl": wdrl,
            "xah": _to_dr(np.asarray(ah)),
            "xal": _to_dr(np.asarray(al)),
            "wt": wt_bf,
            "bd8": bd8,
            "bmat": bmat,
            "bpr": bpr,
        }
        if gamma != 0.0:
            m["xbf"] = np.ascontiguousarray(xi.T).astype(BF)
        in_maps.append(m)
    return in_maps


LAST_RESULT = None


def kernel(x, pos, W, b, gamma, attn_gamma, sum_gamma0, sum_gamma1):
    global LAST_RESULT
    import sys
    sys.path.insert(0, "/opt/trn_rl_repo")
    from concourse.bass_utils import run_bass_kernel_spmd

    x = np.asarray(x, dtype=np.float32)
    pos = np.asarray(pos, dtype=np.float32)
    W = np.asarray(W, dtype=np.float32)
    b = np.asarray(b, dtype=np.float32)
    gamma = float(np.asarray(gamma))
    attn_gamma = float(np.asarray(attn_gamma))
    g0 = math.exp(float(np.asarray(sum_gamma0)))
    g1 = math.exp(float(np.asarray(sum_gamma1)))
    w0, w1 = g0 / (g0 + g1), g1 / (g0 + g1)
    logit_scale = math.sqrt(HD) / attn_gamma

    nc = _build(gamma, w0, w1, logit_scale)
    in_maps = _host_prep(x, pos, W, b, gamma, w0, w1)
    res = run_bass_kernel_spmd(
        nc, in_maps, core_ids=list(range(B)),
        trace=os.environ.get("BK_TRACE", "0") == "1",
    )
    LAST_RESULT = res
    out = np.empty((B, N, D), dtype=np.float32)
    for i in range(B):
        out[i] = res.results[i]["out"].T
    return out
